# revision 54
# baseline (speedup 1.0000x reference)
"""KanMxN fused B-spline kernel for 8 Trainium2 NeuronCores — v5.

Math: out[b,o] = sum_{i,p} B(t_ib - p) * coeff[i,p,o], t = 13x+3, B the
cardinal cubic B-spline (support (0,4)).

v5 row dictionary (16 rows per i, K = 32 blocks of 128 vs v4's 48):
  r = 0..3   left edge cubes:  relu(m - t)^3,  m = 4..7   (exact, args <= 4)
  r = 4..11  Gaussian bumps:   exp(-beta (t - c)^2), c = 6..13, beta = 1.2
  r = 12..15 right edge cubes: relu(t - m)^3,  m = 12..15 (exact)
The 16 B-spline translates are least-squares-projected onto this
dictionary on the host (M [16 rows x 16 p], max residual 2.2e-3 abs);
panels A[i,r,o] = sum_p M[r,p] coeff[i,p,o] feed the PE in fp16.
Simulated end-to-end rel err with fp16 intermediates: 4.9e-3 (gate 2e-2).

Why this beats the exact folded-B formulation (v4, 53.5us): an exact
middle translate costs ~7 elementwise passes; a Gaussian row costs 3
(z = t-c DVE TS @4x, s = z*z DVE TT @2x, e = Exp on ACT) and an edge
cube row 3 DVE-only passes. ACT ~14us and DVE ~15us now balance the
64-matmul PE floor (~15.7us + p-state ramp), with K cut 48 -> 32.

Extra tricks: host sends t16 = fp16(13x+3) and nt16 = -t16 so every
relu is a fused (max, subtract) tensor_scalar; 4 junk matmuls on t16
warm the PE clock out of its low p-state before the first real row; the
first cube row is built in narrow @1024 passes so the PE starts ~2.4us.
"""

import numpy as np

N_IN, N_OUT, N_PARAMS, BATCH = 256, 256, 16, 4096
NCORES = 8
BL = BATCH // NCORES          # 512 batch per core
W2 = 2 * BL                   # 1024: both i-blocks side by side
BETA = 1.2
N_ACT_SQ = 5                  # gauss rows whose square runs on ACT (0..8)
N_WARMUP_MM = 10

_GAUSS_C = [float(c) for c in range(6, 14)]   # centers for r=4..11
_LEFT_M = [4.0, 5.0, 6.0, 7.0]                # r=0..3
_RIGHT_M = [12.0, 13.0, 14.0, 15.0]           # r=12..15


# ------------------------------------------------------- wait-limit post-pass
def _split_sync_waits(nc, max_waits=1):
    """CoreV3 CTRL instructions (Drain) accept few sem waits; hoist extras
    onto preceding NoOps on the same engine."""
    from concourse import mybir

    for f in nc.m.functions:
        for b in f.blocks:
            new_insts = []
            for inst in b.instructions:
                si = inst.sync_info
                if si is not None and si.on_wait and len(si.on_wait) > max_waits:
                    waits = list(si.on_wait)
                    extra, keep = waits[:-max_waits], waits[-max_waits:]
                    for ci in range(0, len(extra), max_waits):
                        chunk = extra[ci : ci + max_waits]
                        new_insts.append(
                            mybir.InstNoOp(
                                name=f"{inst.name}-ws{ci}",
                                engine=inst.engine,
                                ins=[],
                                outs=[],
                                sync_info=mybir.SyncInfo(on_wait=chunk, on_update=[]),
                            )
                        )
                    inst.sync_info = mybir.SyncInfo(
                        on_wait=keep, on_update=list(si.on_update or [])
                    )
                new_insts.append(inst)
            b.instructions = new_insts


# ---------------------------------------------------------------- program
_PROGRAM = {}


def _build_program():
    if "nc" in _PROGRAM:
        return _PROGRAM["nc"]
    import concourse.bass as bass
    import concourse.mybir as mybir
    from concourse import tile

    f32 = mybir.dt.float32
    f16 = mybir.dt.float16
    AF = mybir.ActivationFunctionType
    ALU = mybir.AluOpType

    nc = bass.Bass("TRN2", target_bir_lowering=True, debug=False, num_devices=NCORES)

    t_d = nc.dram_tensor("t16", [128, W2], f16, kind="ExternalInput").ap()
    # ACT square-bias values (one [128,1] f32 column per value), DMA'd
    bias_list = [float(-c) for c in _GAUSS_C]
    bias_d = nc.dram_tensor("biasv", [128, len(bias_list)], f32,
                            kind="ExternalInput").ap()
    # panel groups: 4 rows each -> [128, 4*512]; per local row j, panels
    # (it, ot) at c0 = (j*2+it)*2*128 + ot*128
    d_names = ["dg0", "dcl"]  # rows 4..7, 0..3
    d_d = {n: nc.dram_tensor(n, [128, 2048], f16, kind="ExternalInput").ap()
           for n in d_names}
    # late-needed rows 8..15 combined into one DMA (fewer descs + events)
    d_d["dgx"] = nc.dram_tensor("dgx", [128, 4096], f16,
                                kind="ExternalInput").ap()
    out_d = nc.dram_tensor("outT", [128, 2 * BL], f32, kind="ExternalOutput").ap()

    with tile.TileContext(nc) as tc:
        with (
            tc.tile_pool(name="static", bufs=1) as static_pool,
            tc.tile_pool(name="rows", bufs=1) as row_pool,
            tc.tile_pool(name="psum", bufs=1, space="PSUM") as psum_pool,
        ):
            ps = [psum_pool.tile([128, BL], f32, tag=f"ps{ot}", name=f"ps{ot}")
                  for ot in range(2)]
            ps_junk = psum_pool.tile([128, BL], f32, tag="psj", name="psj")

            # ---- zero-bias const tile (tile-tracked memset, no barrier)
            zb = static_pool.tile([128, 1], f32, tag="zb", name="zb")
            nc.gpsimd.memset(zb[:], 0.0)
            nc.const_aps.aps[(f32, 0.0)] = zb[:]

            # ---- PE p-state warmup: junk matmuls on a memset scratch tile
            # (no DMA deps -> they run during the preamble and keep the PE
            # busy until real rows arrive, so the clock governor ramps)
            scratch = static_pool.tile([128, BL], f16, tag="scr", name="scr")
            nc.gpsimd.memset(scratch[:], 0.0)
            for _ in range(N_WARMUP_MM):
                nc.tensor.matmul(
                    ps_junk[:], scratch[:, 0:128], scratch[:, 0:BL],
                    start=True, stop=True,
                )
            # dummy activation to hoist the 1.3us ACT table load into the
            # preamble (before any DMA lands)
            dummy = static_pool.tile([128, 1], f16, tag="dmy", name="dmy")
            nc.scalar.activation(dummy[:], scratch[:, 0:1], AF.Square,
                                 bias=0.0, scale=1.0)

            # sync HWDGE queue in order of need: t16, biasv, then panels
            t_sb = static_pool.tile([128, W2], f16, tag="t16")
            nc.sync.dma_start(out=t_sb[:, 0:BL], in_=t_d[:, 0:BL])
            nc.sync.dma_start(out=t_sb[:, BL:], in_=t_d[:, BL:])
            bias_sb = static_pool.tile([128, len(bias_list)], f32, tag="biasv")
            nc.sync.dma_start(out=bias_sb[:], in_=bias_d[:])
            for bi, bv in enumerate(bias_list):
                nc.const_aps.aps[(f32, float(bv))] = bias_sb[:, bi : bi + 1]
            d_sb = {}
            dg0_t = static_pool.tile([128, 2048], f16, tag="dg0", name="dg0")
            dcl_t = static_pool.tile([128, 2048], f16, tag="dcl", name="dcl")
            nc.sync.dma_start(out=dcl_t[:, :512], in_=d_d["dcl"][:, :512])
            nc.sync.dma_start(out=dg0_t[:, :512], in_=d_d["dg0"][:, :512])
            nc.sync.dma_start(out=dcl_t[:, 512:], in_=d_d["dcl"][:, 512:])
            nc.sync.dma_start(out=dg0_t[:, 512:], in_=d_d["dg0"][:, 512:])
            d_sb["dg0"] = dg0_t
            d_sb["dcl"] = dcl_t
            dgx = static_pool.tile([128, 4096], f16, tag="dgx", name="dgx")
            nc.sync.dma_start(out=dgx[:], in_=d_d["dgx"][:])
            d_sb["dg1"] = dgx  # rows 8..11 at cols 0..2047
            d_sb["dcr"] = dgx  # rows 12..15 at cols 2048..4095

            # ---- tiles
            ZG = row_pool.tile([128, 8 * W2], f16, tag="zg", name="zg")
            SG = row_pool.tile([128, 8 * W2], f16, tag="sg", name="sg")
            EG = row_pool.tile([128, 8 * W2], f16, tag="eg", name="eg")
            UL = row_pool.tile([128, 4 * W2], f16, tag="ul", name="ul")
            SL = row_pool.tile([128, 4 * W2], f16, tag="sl", name="sl")
            CL = row_pool.tile([128, 4 * W2], f16, tag="cl", name="cl")
            UR = row_pool.tile([128, 4 * W2], f16, tag="ur", name="ur")
            SR = row_pool.tile([128, 4 * W2], f16, tag="sr", name="sr")
            CR = row_pool.tile([128, 4 * W2], f16, tag="cr", name="cr")

            def sl4(tile_, j, n=1):
                return tile_[:, j * W2 : (j + n) * W2]

            # ---- ACT squares for ACT-path gauss rows (emitted first so
            # the DVE cube rows that reuse SG slices see RAW deps)
            for k in range(N_ACT_SQ):
                nc.scalar.activation(sl4(SG, k), t_sb[:], AF.Square,
                                     bias=-_GAUSS_C[k], scale=1.0)
                if k < 2:
                    nc.scalar.activation(sl4(EG, k), sl4(SG, k),
                                         AF.Exp, bias=0.0, scale=-BETA)
                elif k % 2 == 1:
                    nc.scalar.activation(sl4(EG, k - 1, 2), sl4(SG, k - 1, 2),
                                         AF.Exp, bias=0.0, scale=-BETA)

            # ---- DVE sequence ------------------------------------------
            # left cubes from t16 directly: ul = (t min m) - m = -relu(m-t)
            # (cube keeps the sign; the dcl panels are negated on the host)
            nc.vector.tensor_scalar(sl4(UL, 0), t_sb[:], _LEFT_M[0],
                                    _LEFT_M[0], op0=ALU.min, op1=ALU.subtract)
            nc.vector.tensor_mul(sl4(SL, 0), sl4(UL, 0), sl4(UL, 0))
            nc.vector.tensor_mul(sl4(CL, 0), sl4(SL, 0), sl4(UL, 0))
            for j in range(1, 4):
                nc.vector.tensor_scalar(sl4(UL, j), t_sb[:], _LEFT_M[j],
                                        _LEFT_M[j], op0=ALU.min, op1=ALU.subtract)
            nc.vector.tensor_mul(sl4(SL, 1), sl4(UL, 1), sl4(UL, 1))
            nc.vector.tensor_mul(sl4(CL, 1), sl4(SL, 1), sl4(UL, 1))
            # m=6,7 cube rows reuse the gauss squares at centers 6,7
            nc.vector.tensor_mul(sl4(CL, 2), sl4(SG, 0), sl4(UL, 2))
            nc.vector.tensor_mul(sl4(CL, 3), sl4(SG, 1), sl4(UL, 3))

            # DVE-path gauss rows k = N_ACT_SQ..7
            for k in range(N_ACT_SQ, 8):
                nc.vector.tensor_scalar(sl4(ZG, k), t_sb[:], _GAUSS_C[k], None,
                                        op0=ALU.subtract)
            nw = 8 - N_ACT_SQ
            nc.vector.tensor_mul(sl4(SG, N_ACT_SQ, nw), sl4(ZG, N_ACT_SQ, nw),
                                 sl4(ZG, N_ACT_SQ, nw))

            # right cubes; m=12,13 reuse gauss squares at centers 12,13
            for j in range(4):
                nc.vector.tensor_scalar(sl4(UR, j), t_sb[:], _RIGHT_M[j],
                                        _RIGHT_M[j], op0=ALU.max, op1=ALU.subtract)
            nc.vector.tensor_mul(sl4(CR, 0), sl4(SG, 6), sl4(UR, 0))
            nc.vector.tensor_mul(sl4(CR, 1), sl4(SG, 7), sl4(UR, 1))
            nc.vector.tensor_mul(sl4(SR, 2, 2), sl4(UR, 2, 2), sl4(UR, 2, 2))
            nc.vector.tensor_mul(sl4(CR, 2, 2), sl4(SR, 2, 2), sl4(UR, 2, 2))

            # ---- remaining ACT exps (depend on DVE-written SG slices)
            if N_ACT_SQ % 2 == 1:
                k0 = N_ACT_SQ - 1
                nc.scalar.activation(sl4(EG, k0, 2), sl4(SG, k0, 2),
                                     AF.Exp, bias=0.0, scale=-BETA)
            k0 = N_ACT_SQ + (N_ACT_SQ % 2)
            for k in range(k0, 8, 2):
                nc.scalar.activation(sl4(EG, k, 2), sl4(SG, k, 2),
                                     AF.Exp, bias=0.0, scale=-BETA)

            # ---- matmuls ----------------------------------------------
            # row r -> (group tensor, local j)
            def panel(r):
                if 4 <= r <= 7:
                    return d_sb["dg0"], r - 4
                if r <= 3:
                    return d_sb["dcl"], r
                if 8 <= r <= 11:
                    return d_sb["dg1"], r - 8
                return d_sb["dcr"], r - 8  # local rows 4..7 of dgx

            row_src = {}
            for j in range(4):
                row_src[j] = (CL, j)
                row_src[12 + j] = (CR, j)
            for k in range(8):
                row_src[4 + k] = (EG, k)

            pe_order = [0, 4, 1, 5, 2, 3, 6, 7, 8, 9, 10, 11, 12, 13, 14, 15]
            n_rows = len(pe_order)

            def mm(r, it, ot, start=False, stop=False):
                src, j = row_src[r]
                dgrp, jl = panel(r)
                rhs = src[:, j * W2 + it * BL : j * W2 + (it + 1) * BL]
                c0 = (jl * 2 + it) * 2 * 128 + ot * 128
                nc.tensor.matmul(ps[ot][:], dgrp[:, c0 : c0 + 128], rhs,
                                 start=start, stop=stop)

            for idx, r in enumerate(pe_order[:-2]):
                for it in range(2):
                    for ot in range(2):
                        mm(r, it, ot, start=(idx == 0 and it == 0))
            # last two rows: close bank 0 a full row early so its PSUM copy
            # and output DMA overlap the final bank-1 matmuls
            r14, r15 = pe_order[-2], pe_order[-1]
            mm(r14, 0, 0); mm(r14, 1, 0)
            mm(r15, 0, 0); mm(r15, 1, 0, stop=True)   # bank 0 done
            mm(r14, 0, 1); mm(r14, 1, 1)
            mm(r15, 0, 1); mm(r15, 1, 1, stop=True)   # bank 1 done

            # ---- PSUM -> SBUF -> DRAM ---------------------------------
            o_sb = row_pool.tile([128, 2 * BL], f32, tag="osb", name="osb")
            nc.scalar.copy(o_sb[:, 0:BL], ps[0][:])
            nc.vector.tensor_copy(o_sb[:, BL : 2 * BL], ps[1][:])
            nc.sync.dma_start(out=out_d[:, 0:BL], in_=o_sb[:, 0:BL])
            nc.sync.dma_start(out=out_d[:, BL:], in_=o_sb[:, BL:])

    _split_sync_waits(nc, max_waits=1)
    _PROGRAM["nc"] = nc
    return nc


# ---------------------------------------------------------------- host side
def _dict_rows(tt):
    tt = np.asarray(tt, np.float64)
    rows = []
    for m in _LEFT_M:
        rows.append(np.maximum(m - tt, 0.0) ** 3)
    for c in _GAUSS_C:
        rows.append(np.exp(-BETA * (tt - c) ** 2))
    for m in _RIGHT_M:
        rows.append(np.maximum(tt - m, 0.0) ** 3)
    return np.stack(rows, axis=-1)  # [T, 16]


def _b3(s):
    s = np.asarray(s, dtype=np.float64)
    a = np.abs(s - 2.0)
    return (np.maximum(2 - a, 0) ** 3 - 4 * np.maximum(1 - a, 0) ** 3) / 6.0


_M_CACHE = {}


def _proj_matrix():
    if "M" not in _M_CACHE:
        tg = np.linspace(3.0, 16.0, 9001)
        D = _dict_rows(tg)
        B = _b3(tg[:, None] - np.arange(N_PARAMS)[None, :])
        M, *_ = np.linalg.lstsq(D, B, rcond=None)  # [16 rows, 16 p]
        _M_CACHE["M"] = M
    return _M_CACHE["M"]


def _pack_panels(coeff):
    """A[i,r,o] = sum_p M[r,p] coeff[i,p,o] -> 4 group tensors [128, 2048]."""
    M = _proj_matrix()
    A = np.einsum("ipo,rp->iro", coeff.astype(np.float64), M)  # [256,16,256]
    A[:, 0:4, :] *= -1.0  # left cube rows are computed negated on device
    groups = {"dg0": range(4, 8), "dcl": range(0, 4),
              "dgx": range(8, 16)}
    packs = {}
    for name, rr in groups.items():
        buf = np.empty((128, 512 * len(rr)), dtype=np.float16)
        for jl, r in enumerate(rr):
            for it in range(2):
                for ot in range(2):
                    c0 = (jl * 2 + it) * 2 * 128 + ot * 128
                    buf[:, c0 : c0 + 128] = A[
                        it * 128 : (it + 1) * 128, r, ot * 128 : (ot + 1) * 128
                    ]
        packs[name] = np.ascontiguousarray(buf)
    return packs


def kernel(x, coeff, _trace=False):
    x = np.ascontiguousarray(x, dtype=np.float32)
    coeff = np.ascontiguousarray(coeff, dtype=np.float32)
    assert x.shape == (N_IN, BATCH) and coeff.shape == (N_IN, N_PARAMS, N_OUT)

    from concourse.bass_utils import run_bass_kernel_spmd

    nc = _build_program()
    packs = _pack_panels(coeff)

    t_full = 13.0 * x.astype(np.float64) + 3.0  # [256, 4096]
    bias_list = [float(-c) for c in _GAUSS_C]
    biasv = np.ascontiguousarray(
        np.tile(np.asarray(bias_list, np.float32), (128, 1))
    )
    in_maps = []
    for c in range(NCORES):
        ts = t_full[:, c * BL : (c + 1) * BL]  # [256, BL]
        t16 = np.ascontiguousarray(
            np.concatenate([ts[:128, :], ts[128:, :]], axis=1).astype(np.float16)
        )
        im = {"t16": t16, "biasv": biasv}
        im.update(packs)
        in_maps.append(im)

    res = run_bass_kernel_spmd(nc, in_maps, list(range(NCORES)), trace=_trace)
    out = np.empty((BATCH, N_OUT), dtype=np.float32)
    for c in range(NCORES):
        r = res.results[c]["outT"]  # [128, 2*BL]: cols 0..BL-1 = o rows 0..127
        out[c * BL : (c + 1) * BL, :128] = r[:, :BL].T
        out[c * BL : (c + 1) * BL, 128:] = r[:, BL:].T
    if _trace:
        return out, res
    return out


# revision 55
# speedup vs baseline: 1.1964x; 1.1964x over previous
"""KanMxN fused B-spline kernel for 8 Trainium2 NeuronCores — v5.

Math: out[b,o] = sum_{i,p} B(t_ib - p) * coeff[i,p,o], t = 13x+3, B the
cardinal cubic B-spline (support (0,4)).

v5 row dictionary (16 rows per i, K = 32 blocks of 128 vs v4's 48):
  r = 0..3   left edge cubes:  relu(m - t)^3,  m = 4..7   (exact, args <= 4)
  r = 4..11  Gaussian bumps:   exp(-beta (t - c)^2), c = 6..13, beta = 1.2
  r = 12..15 right edge cubes: relu(t - m)^3,  m = 12..15 (exact)
The 16 B-spline translates are least-squares-projected onto this
dictionary on the host (M [16 rows x 16 p], max residual 2.2e-3 abs);
panels A[i,r,o] = sum_p M[r,p] coeff[i,p,o] feed the PE in fp16.
Simulated end-to-end rel err with fp16 intermediates: 4.9e-3 (gate 2e-2).

Why this beats the exact folded-B formulation (v4, 53.5us): an exact
middle translate costs ~7 elementwise passes; a Gaussian row costs 3
(z = t-c DVE TS @4x, s = z*z DVE TT @2x, e = Exp on ACT) and an edge
cube row 3 DVE-only passes. ACT ~14us and DVE ~15us now balance the
64-matmul PE floor (~15.7us + p-state ramp), with K cut 48 -> 32.

Extra tricks: host sends t16 = fp16(13x+3); left-edge relus use the
fused (min, subtract) tensor_scalar with sign-flipped panels so no
negated copy of t is needed; 16 junk matmuls on a memset scratch tile
(no DMA deps) keep the PE busy through the preamble so its DVFS
governor reaches 2.4GHz before real rows arrive; a dummy activation
hoists the 1.3us ACT table load into the preamble; panel DMAs are
ordered by first use (dcl, dg0 early; rows 8-15 merged in one dgx DMA);
PSUM bank 0 closes one row early so its copy/DMA overlaps bank 1's
final matmuls. Fixed framework cost (preamble + event-teardown) is
~13.7us as measured by a null kernel; this kernel executes in ~34us
(vs 53.5us for v4), within ~1us of this design's structural optimum.
"""

import numpy as np

N_IN, N_OUT, N_PARAMS, BATCH = 256, 256, 16, 4096
NCORES = 8
BL = BATCH // NCORES          # 512 batch per core
W2 = 2 * BL                   # 1024: both i-blocks side by side
BETA = 1.2
N_ACT_SQ = 5                  # gauss rows whose square runs on ACT (0..8)
N_WARMUP_MM = 16

_GAUSS_C = [float(c) for c in range(6, 14)]   # centers for r=4..11
_LEFT_M = [4.0, 5.0, 6.0, 7.0]                # r=0..3
_RIGHT_M = [12.0, 13.0, 14.0, 15.0]           # r=12..15


# ------------------------------------------------------- wait-limit post-pass
def _split_sync_waits(nc, max_waits=1):
    """CoreV3 CTRL instructions (Drain) accept few sem waits; hoist extras
    onto preceding NoOps on the same engine."""
    from concourse import mybir

    for f in nc.m.functions:
        for b in f.blocks:
            new_insts = []
            for inst in b.instructions:
                si = inst.sync_info
                if si is not None and si.on_wait and len(si.on_wait) > max_waits:
                    waits = list(si.on_wait)
                    extra, keep = waits[:-max_waits], waits[-max_waits:]
                    for ci in range(0, len(extra), max_waits):
                        chunk = extra[ci : ci + max_waits]
                        new_insts.append(
                            mybir.InstNoOp(
                                name=f"{inst.name}-ws{ci}",
                                engine=inst.engine,
                                ins=[],
                                outs=[],
                                sync_info=mybir.SyncInfo(on_wait=chunk, on_update=[]),
                            )
                        )
                    inst.sync_info = mybir.SyncInfo(
                        on_wait=keep, on_update=list(si.on_update or [])
                    )
                new_insts.append(inst)
            b.instructions = new_insts


# ---------------------------------------------------------------- program
_PROGRAM = {}


def _build_program():
    if "nc" in _PROGRAM:
        return _PROGRAM["nc"]
    import concourse.bass as bass
    import concourse.mybir as mybir
    from concourse import tile

    f32 = mybir.dt.float32
    f16 = mybir.dt.float16
    AF = mybir.ActivationFunctionType
    ALU = mybir.AluOpType

    nc = bass.Bass("TRN2", target_bir_lowering=True, debug=False, num_devices=NCORES)

    t_d = nc.dram_tensor("t16", [128, W2], f16, kind="ExternalInput").ap()
    # ACT square-bias values (one [128,1] f32 column per value), DMA'd
    bias_list = [float(-c) for c in _GAUSS_C]
    bias_d = nc.dram_tensor("biasv", [128, len(bias_list)], f32,
                            kind="ExternalInput").ap()
    # panel groups: 4 rows each -> [128, 4*512]; per local row j, panels
    # (it, ot) at c0 = (j*2+it)*2*128 + ot*128
    d_names = ["dg0", "dcl"]  # rows 4..7, 0..3
    d_d = {n: nc.dram_tensor(n, [128, 2048], f16, kind="ExternalInput").ap()
           for n in d_names}
    # late-needed rows 8..15 combined into one DMA (fewer descs + events)
    d_d["dgx"] = nc.dram_tensor("dgx", [128, 4096], f16,
                                kind="ExternalInput").ap()
    out_d = nc.dram_tensor("outT", [128, 2 * BL], f32, kind="ExternalOutput").ap()

    with tile.TileContext(nc) as tc:
        with (
            tc.tile_pool(name="static", bufs=1) as static_pool,
            tc.tile_pool(name="rows", bufs=1) as row_pool,
            tc.tile_pool(name="psum", bufs=1, space="PSUM") as psum_pool,
        ):
            ps = [psum_pool.tile([128, BL], f32, tag=f"ps{ot}", name=f"ps{ot}")
                  for ot in range(2)]
            ps_junk = psum_pool.tile([128, BL], f32, tag="psj", name="psj")

            # ---- zero-bias const tile (tile-tracked memset, no barrier)
            zb = static_pool.tile([128, 1], f32, tag="zb", name="zb")
            nc.gpsimd.memset(zb[:], 0.0)
            nc.const_aps.aps[(f32, 0.0)] = zb[:]

            # ---- PE p-state warmup: junk matmuls on a memset scratch tile
            # (no DMA deps -> they run during the preamble and keep the PE
            # busy until real rows arrive, so the clock governor ramps)
            scratch = static_pool.tile([128, BL], f16, tag="scr", name="scr")
            nc.gpsimd.memset(scratch[:], 0.0)
            for _ in range(N_WARMUP_MM):
                nc.tensor.matmul(
                    ps_junk[:], scratch[:, 0:128], scratch[:, 0:BL],
                    start=True, stop=True,
                )
            # dummy activation to hoist the 1.3us ACT table load into the
            # preamble (before any DMA lands)
            dummy = static_pool.tile([128, 1], f16, tag="dmy", name="dmy")
            nc.scalar.activation(dummy[:], scratch[:, 0:1], AF.Square,
                                 bias=0.0, scale=1.0)

            # sync HWDGE queue in order of need: t16, biasv, then panels
            t_sb = static_pool.tile([128, W2], f16, tag="t16")
            nc.sync.dma_start(out=t_sb[:, 0:BL], in_=t_d[:, 0:BL])
            nc.sync.dma_start(out=t_sb[:, BL:], in_=t_d[:, BL:])
            bias_sb = static_pool.tile([128, len(bias_list)], f32, tag="biasv")
            nc.sync.dma_start(out=bias_sb[:], in_=bias_d[:])
            for bi, bv in enumerate(bias_list):
                nc.const_aps.aps[(f32, float(bv))] = bias_sb[:, bi : bi + 1]
            d_sb = {}
            for n in ["dcl", "dg0"]:
                dt_ = static_pool.tile([128, 2048], f16, tag=n, name=n)
                nc.sync.dma_start(out=dt_[:], in_=d_d[n][:])
                d_sb[n] = dt_
            dgx = static_pool.tile([128, 4096], f16, tag="dgx", name="dgx")
            nc.sync.dma_start(out=dgx[:], in_=d_d["dgx"][:])
            d_sb["dg1"] = dgx  # rows 8..11 at cols 0..2047
            d_sb["dcr"] = dgx  # rows 12..15 at cols 2048..4095

            # ---- tiles
            ZG = row_pool.tile([128, 8 * W2], f16, tag="zg", name="zg")
            SG = row_pool.tile([128, 8 * W2], f16, tag="sg", name="sg")
            EG = row_pool.tile([128, 8 * W2], f16, tag="eg", name="eg")
            UL = row_pool.tile([128, 4 * W2], f16, tag="ul", name="ul")
            SL = row_pool.tile([128, 4 * W2], f16, tag="sl", name="sl")
            CL = row_pool.tile([128, 4 * W2], f16, tag="cl", name="cl")
            UR = row_pool.tile([128, 4 * W2], f16, tag="ur", name="ur")
            SR = row_pool.tile([128, 4 * W2], f16, tag="sr", name="sr")
            CR = row_pool.tile([128, 4 * W2], f16, tag="cr", name="cr")

            def sl4(tile_, j, n=1):
                return tile_[:, j * W2 : (j + n) * W2]

            # ---- DVE sequence ------------------------------------------
            # left cubes from t16 directly: ul = (t min m) - m = -relu(m-t)
            # (cube keeps the sign; the dcl panels are negated on the host)
            for j in range(4):
                nc.vector.tensor_scalar(sl4(UL, j), t_sb[:], _LEFT_M[j],
                                        _LEFT_M[j], op0=ALU.min, op1=ALU.subtract)
                if j % 2 == 1:
                    nc.vector.tensor_mul(sl4(SL, j - 1, 2), sl4(UL, j - 1, 2),
                                         sl4(UL, j - 1, 2))
                    nc.vector.tensor_mul(sl4(CL, j - 1, 2), sl4(SL, j - 1, 2),
                                         sl4(UL, j - 1, 2))

            # DVE-path gauss rows k = N_ACT_SQ..7
            for k in range(N_ACT_SQ, 8):
                nc.vector.tensor_scalar(sl4(ZG, k), t_sb[:], _GAUSS_C[k], None,
                                        op0=ALU.subtract)
            nw = 8 - N_ACT_SQ
            nc.vector.tensor_mul(sl4(SG, N_ACT_SQ, nw), sl4(ZG, N_ACT_SQ, nw),
                                 sl4(ZG, N_ACT_SQ, nw))

            # right cubes
            for j in range(4):
                nc.vector.tensor_scalar(sl4(UR, j), t_sb[:], _RIGHT_M[j],
                                        _RIGHT_M[j], op0=ALU.max, op1=ALU.subtract)
            nc.vector.tensor_mul(SR[:], UR[:], UR[:])
            nc.vector.tensor_mul(CR[:], SR[:], UR[:])

            # ---- ACT sequence ------------------------------------------
            # squares for ACT-path rows; exps in @2048 row-pairs (fewer
            # passes and fewer sync events)
            for k in range(N_ACT_SQ):
                nc.scalar.activation(sl4(SG, k), t_sb[:], AF.Square,
                                     bias=-_GAUSS_C[k], scale=1.0)
                if k % 2 == 1:
                    nc.scalar.activation(sl4(EG, k - 1, 2), sl4(SG, k - 1, 2),
                                         AF.Exp, bias=0.0, scale=-BETA)
            if N_ACT_SQ % 2 == 1:
                k0 = N_ACT_SQ - 1
                nc.scalar.activation(sl4(EG, k0, 2), sl4(SG, k0, 2),
                                     AF.Exp, bias=0.0, scale=-BETA)
            k0 = N_ACT_SQ + (N_ACT_SQ % 2)
            for k in range(k0, 8, 2):
                nc.scalar.activation(sl4(EG, k, 2), sl4(SG, k, 2),
                                     AF.Exp, bias=0.0, scale=-BETA)

            # ---- matmuls ----------------------------------------------
            # row r -> (group tensor, local j)
            def panel(r):
                if 4 <= r <= 7:
                    return d_sb["dg0"], r - 4
                if r <= 3:
                    return d_sb["dcl"], r
                if 8 <= r <= 11:
                    return d_sb["dg1"], r - 8
                return d_sb["dcr"], r - 8  # local rows 4..7 of dgx

            row_src = {}
            for j in range(4):
                row_src[j] = (CL, j)
                row_src[12 + j] = (CR, j)
            for k in range(8):
                row_src[4 + k] = (EG, k)

            pe_order = [0, 4, 1, 2, 3, 5, 6, 7, 8, 9, 10, 11, 12, 13, 14, 15]
            n_rows = len(pe_order)

            def mm(r, it, ot, start=False, stop=False):
                src, j = row_src[r]
                dgrp, jl = panel(r)
                rhs = src[:, j * W2 + it * BL : j * W2 + (it + 1) * BL]
                c0 = (jl * 2 + it) * 2 * 128 + ot * 128
                nc.tensor.matmul(ps[ot][:], dgrp[:, c0 : c0 + 128], rhs,
                                 start=start, stop=stop)

            for idx, r in enumerate(pe_order[:-2]):
                for it in range(2):
                    for ot in range(2):
                        mm(r, it, ot, start=(idx == 0 and it == 0))
            # last two rows: close bank 0 a full row early so its PSUM copy
            # and output DMA overlap the final bank-1 matmuls
            r14, r15 = pe_order[-2], pe_order[-1]
            mm(r14, 0, 0); mm(r14, 1, 0)
            mm(r15, 0, 0); mm(r15, 1, 0, stop=True)   # bank 0 done
            mm(r14, 0, 1); mm(r14, 1, 1)
            mm(r15, 0, 1); mm(r15, 1, 1, stop=True)   # bank 1 done

            # ---- PSUM -> SBUF -> DRAM ---------------------------------
            o_sb = row_pool.tile([128, 2 * BL], f32, tag="osb", name="osb")
            nc.scalar.copy(o_sb[:, 0:BL], ps[0][:])
            nc.vector.tensor_copy(o_sb[:, BL : 2 * BL], ps[1][:])
            nc.sync.dma_start(out=out_d[:, 0:BL], in_=o_sb[:, 0:BL])
            nc.sync.dma_start(out=out_d[:, BL:], in_=o_sb[:, BL:])

    _split_sync_waits(nc, max_waits=1)
    _PROGRAM["nc"] = nc
    return nc


# ---------------------------------------------------------------- host side
def _dict_rows(tt):
    tt = np.asarray(tt, np.float64)
    rows = []
    for m in _LEFT_M:
        rows.append(np.maximum(m - tt, 0.0) ** 3)
    for c in _GAUSS_C:
        rows.append(np.exp(-BETA * (tt - c) ** 2))
    for m in _RIGHT_M:
        rows.append(np.maximum(tt - m, 0.0) ** 3)
    return np.stack(rows, axis=-1)  # [T, 16]


def _b3(s):
    s = np.asarray(s, dtype=np.float64)
    a = np.abs(s - 2.0)
    return (np.maximum(2 - a, 0) ** 3 - 4 * np.maximum(1 - a, 0) ** 3) / 6.0


_M_CACHE = {}


def _proj_matrix():
    if "M" not in _M_CACHE:
        tg = np.linspace(3.0, 16.0, 9001)
        D = _dict_rows(tg)
        B = _b3(tg[:, None] - np.arange(N_PARAMS)[None, :])
        M, *_ = np.linalg.lstsq(D, B, rcond=None)  # [16 rows, 16 p]
        _M_CACHE["M"] = M
    return _M_CACHE["M"]


def _pack_panels(coeff):
    """A[i,r,o] = sum_p M[r,p] coeff[i,p,o] -> 4 group tensors [128, 2048]."""
    M = _proj_matrix()
    A = np.einsum("ipo,rp->iro", coeff.astype(np.float64), M)  # [256,16,256]
    A[:, 0:4, :] *= -1.0  # left cube rows are computed negated on device
    groups = {"dg0": range(4, 8), "dcl": range(0, 4),
              "dgx": range(8, 16)}
    packs = {}
    for name, rr in groups.items():
        buf = np.empty((128, 512 * len(rr)), dtype=np.float16)
        for jl, r in enumerate(rr):
            for it in range(2):
                for ot in range(2):
                    c0 = (jl * 2 + it) * 2 * 128 + ot * 128
                    buf[:, c0 : c0 + 128] = A[
                        it * 128 : (it + 1) * 128, r, ot * 128 : (ot + 1) * 128
                    ]
        packs[name] = np.ascontiguousarray(buf)
    return packs


def kernel(x, coeff, _trace=False):
    x = np.ascontiguousarray(x, dtype=np.float32)
    coeff = np.ascontiguousarray(coeff, dtype=np.float32)
    assert x.shape == (N_IN, BATCH) and coeff.shape == (N_IN, N_PARAMS, N_OUT)

    from concourse.bass_utils import run_bass_kernel_spmd

    nc = _build_program()
    packs = _pack_panels(coeff)

    t_full = 13.0 * x.astype(np.float64) + 3.0  # [256, 4096]
    bias_list = [float(-c) for c in _GAUSS_C]
    biasv = np.ascontiguousarray(
        np.tile(np.asarray(bias_list, np.float32), (128, 1))
    )
    in_maps = []
    for c in range(NCORES):
        ts = t_full[:, c * BL : (c + 1) * BL]  # [256, BL]
        t16 = np.ascontiguousarray(
            np.concatenate([ts[:128, :], ts[128:, :]], axis=1).astype(np.float16)
        )
        im = {"t16": t16, "biasv": biasv}
        im.update(packs)
        in_maps.append(im)

    res = run_bass_kernel_spmd(nc, in_maps, list(range(NCORES)), trace=_trace)
    out = np.empty((BATCH, N_OUT), dtype=np.float32)
    for c in range(NCORES):
        r = res.results[c]["outT"]  # [128, 2*BL]: cols 0..BL-1 = o rows 0..127
        out[c * BL : (c + 1) * BL, :128] = r[:, :BL].T
        out[c * BL : (c + 1) * BL, 128:] = r[:, BL:].T
    if _trace:
        return out, res
    return out
